# revision 1
# baseline (speedup 1.0000x reference)
"""CRF negative-log-likelihood loss kernel for Trainium2 (Bass/Tile).

Strategy (data-parallel over batch, 8 NeuronCores, 32 rows each):
  - log-partition via probability-domain forward scan:
        a_t = exp(x_t + bias) * (E^T a_{t-1}),   E = exp(trans)
    one PE matmul + one DVE multiply per step (2 independent column-chains
    to hide sync latency). Periodic global rescale is folded into the exp
    bias; the cumulative log-offset M is tracked per rescale epoch.
  - full a-history kept in SBUF (bf16); logZ[b] recovered at t=seq_len[b]-1
    by a gpsimd ap_gather column gather (d=2 pairs, parity handled by two
    strided ones-matmuls) + Ln.
  - gold score:
      unary  = sum_t logits[b,t,label] via (iota==label)*logits fused
               multiply-accumulate (DVE scalar_tensor_tensor, accum_out).
      pair   = sum_t trans[lab_t,lab_{t+1}] via ap_gather from a replicated
               flat trans; the sequence mask is folded into the gather index
               (masked slots point at an appended zero entry).
  - per-core partial losses summed on host.
"""

import numpy as np

B, T, K = 256, 512, 128
NCORES = 8
BL = B // NCORES          # 32 batch rows per core
NJ = 4                    # t-chunks for the score layout: partition=(b, j)
TJ = T // NJ              # 128
EPOCH = 8                 # rescale every EPOCH scan steps
NEP = T // EPOCH          # 128 epochs
COMP = 32.0               # expected log-growth per epoch (centering constant)
NCH = 2                   # independent scan chains (column split of the 32 rows)
CW = BL // NCH            # columns per chain

_CACHE = {}


# ----------------------------------------------------------------------------
# host-side constant tables (shape-only, input independent)
# ----------------------------------------------------------------------------
def _host_consts():
    c = {}
    c["c_id32"] = np.eye(32, dtype=np.float32)
    # iota over k along free dim, same for every partition
    c["c_iota_k"] = np.tile(np.arange(K, dtype=np.float32)[None, :], (128, 1))
    # global t for the unary (b,j) layout: t = j*TJ + t_local
    j = (np.arange(128) % NJ)[:, None]
    c["c_tw4"] = (j * TJ + np.arange(TJ)[None, :]).astype(np.float32)
    # t+1 in the pair wrapped layout: slot (p=16g+r, s) -> t = (s*16+r) % 512
    p = np.arange(128)[:, None]
    s = np.arange(128)[None, :]
    c["c_tp1w"] = (((s * 16 + (p % 16)) % 512) + 1).astype(np.int32)
    # b//2 for the [16,2]-wrapped capture index build (replicated to 128)
    pp = np.arange(128)[:, None] % 16
    cc = np.arange(2)[None, :]
    c["c_bhalf"] = ((cc * 16 + pp) // 2).astype(np.int32)
    # unary combine: [128, 32] sel[p, b] = (p//4 == b)
    c["c_comb_u"] = (np.arange(128)[:, None] // NJ == np.arange(32)[None, :]).astype(
        np.float32
    )
    # pair combine, one per b_local: [128, 32] sel[p,b] = (p%16==0 and b==4*(p//16)+bl)
    for bl in range(4):
        m = (np.arange(128)[:, None] % 16 == 0) & (
            np.arange(32)[None, :] == 4 * (np.arange(128)[:, None] // 16) + bl
        )
        c[f"c_comb_p{bl}"] = m.astype(np.float32)
    # L replication helpers (PE partition-permute)
    c["c_selb4"] = (np.arange(32)[:, None] == np.arange(128)[None, :] // 4).astype(
        np.float32
    )
    c["c_sel16"] = (
        np.arange(32)[:, None] // 4 == np.arange(128)[None, :] // 16
    ).astype(np.float32)
    c["c_bsel"] = (np.arange(32)[:, None] % 4 == np.arange(4)[None, :]).astype(
        np.float32
    )
    c["c_onesrow"] = np.ones((1, 128), dtype=np.float32)
    c["c_ones16"] = np.ones((1, 16), dtype=np.float32)
    c["c_onescol"] = np.ones((128, 1), dtype=np.float32)
    return c


# ----------------------------------------------------------------------------
# the Tile program
# ----------------------------------------------------------------------------
def _build_program():
    from contextlib import ExitStack

    import concourse.bass as bass
    import concourse.mybir as mybir
    import concourse.tile as tile
    from concourse import bacc

    f32 = mybir.dt.float32
    bf16 = mybir.dt.bfloat16
    i32 = mybir.dt.int32
    i16 = mybir.dt.int16
    AX = mybir.AxisListType
    OP = mybir.AluOpType
    ACTF = mybir.ActivationFunctionType

    nc = bacc.Bacc("TRN2", target_bir_lowering=False, debug=False)

    logits_d = nc.dram_tensor("logits", [BL, T, K], f32, kind="ExternalInput").ap()
    labels_d = nc.dram_tensor("labels", [BL, T], i32, kind="ExternalInput").ap()
    seq_d = nc.dram_tensor("seq_lens", [BL], i32, kind="ExternalInput").ap()
    trans_d = nc.dram_tensor("trans", [K, K], f32, kind="ExternalInput").ap()
    consts_np = _host_consts()
    cd = {}
    for name, arr in consts_np.items():
        cd[name] = nc.dram_tensor(
            name, list(arr.shape), mybir.dt.from_np(arr.dtype), kind="ExternalInput"
        ).ap()
    loss_d = nc.dram_tensor("loss", [1, 1], f32, kind="ExternalOutput").ap()

    with tile.TileContext(nc) as tc, ExitStack() as ctx:
        const_pool = ctx.enter_context(tc.tile_pool(name="const", bufs=1))
        big_pool = ctx.enter_context(tc.tile_pool(name="big", bufs=1))
        x4_pool = ctx.enter_context(tc.tile_pool(name="x4", bufs=3))
        ex_pool = ctx.enter_context(tc.tile_pool(name="ex", bufs=10))
        small_pool = ctx.enter_context(tc.tile_pool(name="small", bufs=1))
        ev_pool = ctx.enter_context(tc.tile_pool(name="ev", bufs=4))
        ps_xt = ctx.enter_context(tc.tile_pool(name="ps_xt", bufs=3, space="PSUM"))
        ps_u = ctx.enter_context(tc.tile_pool(name="ps_u", bufs=3, space="PSUM"))
        ps_misc = ctx.enter_context(tc.tile_pool(name="ps_misc", bufs=1, space="PSUM"))

        # ---- constants into SBUF ----
        def load_const(name, eng=None):
            ap = cd[name]
            t = const_pool.tile(list(ap.shape), ap.dtype, tag=name)
            (eng or nc.gpsimd).dma_start(t[:], ap[:])
            return t

        id32 = load_const("c_id32", nc.sync)
        onescol_f = load_const("c_onescol", nc.sync)
        onescol_bf = const_pool.tile([128, 1], bf16, tag="onescol_bf16")
        nc.vector.tensor_copy(onescol_bf[:], onescol_f[:])

        # ---- E = exp(trans), bf16 stationary ----
        trs = small_pool.tile([K, K], f32, tag="trs")
        nc.sync.dma_start(trs[:], trans_d[:, :])
        e_bf = const_pool.tile([K, K], bf16, tag="e_bf")
        nc.scalar.activation(e_bf[:], trs[:], ACTF.Exp)

        # labels + prep-critical consts on the Pool queue (in this order)
        lab4 = small_pool.tile([128, TJ], i32, tag="lab4")
        nc.gpsimd.dma_start(lab4[:], labels_d.rearrange("b (j t) -> (b j) t", j=NJ))
        lab_flat = labels_d.rearrange("b t -> (b t)")
        lab_w = small_pool.tile([128, 128], i32, tag="lab_w")
        for g in range(8):
            nc.gpsimd.dma_start(
                lab_w[16 * g : 16 * g + 16, :],
                lab_flat[g * 2048 : (g + 1) * 2048].rearrange(
                    "(s r) -> r s", s=128, r=16
                ),
            )
        labn_w = small_pool.tile([128, 128], i32, tag="labn_w")
        nc.vector.memset(labn_w[:], 0)
        for g in range(7):
            nc.gpsimd.dma_start(
                labn_w[16 * g : 16 * g + 16, :],
                lab_flat[1 + g * 2048 : 1 + (g + 1) * 2048].rearrange(
                    "(s r) -> r s", s=128, r=16
                ),
            )
        nc.gpsimd.dma_start(
            labn_w[112:128, 0:127],
            lab_flat[1 + 7 * 2048 : 1 + 7 * 2048 + 127 * 16].rearrange(
                "(s r) -> r s", s=127, r=16
            ),
        )
        nc.gpsimd.dma_start(
            labn_w[112:127, 127:128],
            lab_flat[1 + 7 * 2048 + 127 * 16 : 16384].rearrange("(r o) -> r o", o=1),
        )
        tp1w = load_const("c_tp1w")
        bsel = load_const("c_bsel")
        sel16 = load_const("c_sel16")
        selb4 = load_const("c_selb4")
        iota_k = load_const("c_iota_k")
        tw4 = load_const("c_tw4")
        onesrow = load_const("c_onesrow")

        # ---- seq_lens loads (emitted into the SP queue early; cheap) ----
        seq32 = small_pool.tile([BL, 1], i32, tag="seq32")
        nc.sync.dma_start(seq32[:], seq_d.rearrange("(b o) -> b o", o=1))
        seqf = small_pool.tile([BL, 1], f32, tag="seqf")
        nc.vector.tensor_copy(seqf[:], seq32[:])
        # wrapped [16,2] copies of seq_lens replicated into [128,2]
        lw128 = small_pool.tile([128, 2], i32, tag="lw128")
        seq_w = seq_d.rearrange("(c p) -> p c", c=2)
        for g in range(8):
            nc.sync.dma_start(lw128[16 * g : 16 * g + 16, :], seq_w)
        bhalf = load_const("c_bhalf", nc.sync)

        # L replicated per unary partition: L_rep4[p] = L[p//4]
        lrep4_ps = ps_misc.tile([128, 1], f32, tag="mm_small")
        nc.tensor.matmul(lrep4_ps[:], selb4[:], seqf[:], start=True, stop=True)
        lrep4 = small_pool.tile([128, 1], f32, tag="lrep4")
        nc.vector.tensor_copy(lrep4[:], lrep4_ps[:])
        # L for the pair wrapped layout: L_blk[p, q] = L[4*(p//16)+q]
        lb4 = small_pool.tile([BL, 4], f32, tag="lb4")
        nc.vector.tensor_scalar(lb4[:], bsel[:], seqf[:], None, OP.mult)
        lblk_ps = ps_misc.tile([128, 4], f32, tag="mm_small")
        nc.tensor.matmul(lblk_ps[:], sel16[:], lb4[:], start=True, stop=True)
        lblk = small_pool.tile([128, 4], f32, tag="lblk")
        nc.vector.tensor_copy(lblk[:], lblk_ps[:])

        # ---- score-phase big loads staggered inside the scan loop ----
        data_u = big_pool.tile([128, TJ * K], f32, tag="data_u")
        src_u = logits_d.rearrange("b (j t) k -> (b j) (t k)", j=NJ)
        # flat trans replicated on every partition, plus a zero slot at 16384
        tr_rep = big_pool.tile([128, K * K + 1], f32, tag="tr_rep")
        tr_flat = trans_d.rearrange("a b -> (a b)")
        nc.vector.memset(tr_rep[:, K * K : K * K + 1], 0.0)

        # unary score prep: masked labels, accumulator (STTs sprinkle in-loop)
        lab4f = small_pool.tile([128, TJ], f32, tag="lab4f")
        nc.vector.tensor_copy(lab4f[:], lab4[:])
        mask_u = small_pool.tile([128, TJ], f32, tag="mask_u")
        nc.vector.tensor_scalar(mask_u[:], tw4[:], lrep4[:], None, OP.is_lt)
        lab4m = small_pool.tile([128, TJ], f32, tag="lab4m")
        nc.vector.scalar_tensor_tensor(
            lab4m[:], lab4f[:], 1.0, mask_u[:], OP.add, OP.mult
        )
        nc.vector.tensor_scalar(lab4m[:], lab4m[:], -1.0, None, OP.add)
        u_acc = small_pool.tile([128, TJ], f32, tag="u_acc")
        junk_u = small_pool.tile([128, K], f32, tag="junk_u")


        # ================= the scan =================
        # probability-domain forward scan, rescaled multiplicatively every
        # EPOCH steps (rc = e^-COMP / colsum of a lagged u row). Logs are
        # deferred: C_hist collects the raw sums, one Ln + prefix-scan at
        # the end produces the per-epoch offset M.
        a_hist = big_pool.tile([128, T * BL], bf16, tag="a_hist")
        c_hist = small_pool.tile([1, NEP], f32, tag="c_hist")
        nc.vector.memset(c_hist[:], 1.0)

        rc_tiles = {}
        u_keep = {}
        x8_tiles = {}
        ex4_tiles = {}
        ECOMP = float(np.exp(COMP))
        for t in range(T):
            m8 = t // 8
            m4 = t // 4
            if t % 8 == 0:
                x8 = x4_pool.tile([BL, 8 * K], f32, tag="x4")
                nc.sync.dma_start(
                    x8[:],
                    logits_d[:, 8 * m8 : 8 * m8 + 8, :].rearrange("b t k -> b (t k)"),
                )
                x8_tiles[m8] = x8
            if t % 4 == 0:
                x8 = x8_tiles[m8]
                xoff = (m4 % 2) * 4 * K
                xt4_ps = ps_xt.tile([K, 4 * BL], f32, tag="xt")
                for i in range(4):
                    nc.tensor.transpose(
                        xt4_ps[:, i * BL : (i + 1) * BL],
                        x8[:, xoff + i * K : xoff + (i + 1) * K],
                        id32[:],
                    )
                ex4 = ex_pool.tile([K, 4 * BL], bf16, tag="ex")
                nc.scalar.activation(ex4[:], xt4_ps[:], ACTF.Exp)
                ex4_tiles[m4] = ex4
            if 16 <= t < 80 and t % 8 == 0:
                r = (t - 16) // 8
                sl = slice(r * 2048, (r + 1) * 2048)
                nc.gpsimd.dma_start(
                    tr_rep[:, sl],
                    bass.AP(tr_flat.tensor, sl.start, [[0, 128], [1, 2048]]),
                )
            if 88 <= t < 152 and t % 8 == 0:
                c = (t - 88) // 8
                sl = slice(c * 2048, (c + 1) * 2048)
                nc.gpsimd.dma_start(data_u[:, sl], src_u[:, sl])
            if t == 82:
                    # pair: mask in wrapped layout -> gather index (masked -> zero slot)
                    mask_w = small_pool.tile([128, 128], i32, tag="mask_w")
                    for q in range(4):
                        nc.vector.tensor_scalar(
                            mask_w[:, q * 32 : (q + 1) * 32],
                            tp1w[:, q * 32 : (q + 1) * 32],
                            lblk[:, q : q + 1],
                            None,
                            OP.is_lt,
                        )
                    pidx = small_pool.tile([128, 128], i32, tag="pidx")
                    nc.vector.tensor_scalar(pidx[:], lab_w[:], 128, None, OP.mult)
                    nc.vector.tensor_tensor(pidx[:], pidx[:], labn_w[:], OP.add)
                    nc.vector.tensor_scalar(pidx[:], pidx[:], -16384, None, OP.add)
                    nc.vector.tensor_tensor(pidx[:], pidx[:], mask_w[:], OP.mult)
                    pidx16 = small_pool.tile([128, 128], i16, tag="pidx16")
                    nc.vector.tensor_scalar(pidx16[:], pidx[:], 16384, None, OP.add)
                    pair_g = small_pool.tile([128, 2048], f32, tag="pair_g")
                    nc.gpsimd.ap_gather(
                        pair_g[:], tr_rep[:], pidx16[:], channels=128, num_elems=K * K + 1, d=1,
                        num_idxs=2048,
                    )

            ex_sl = ex4_tiles[m4][:, (t % 4) * BL : (t % 4 + 1) * BL]
            if t == 0:
                nc.vector.tensor_copy(a_hist[:, 0:BL], ex_sl)
            else:
                up = ps_u.tile([K, BL], f32, tag="u0")
                nc.tensor.matmul(
                    up[:],
                    e_bf[:],
                    a_hist[:, (t - 1) * BL : t * BL],
                    start=True,
                    stop=True,
                )
                if t % EPOCH == 0 and t // EPOCH in rc_tiles:
                    nc.vector.scalar_tensor_tensor(
                        a_hist[:, t * BL : (t + 1) * BL],
                        up[:],
                        rc_tiles[t // EPOCH][:],
                        ex_sl,
                        OP.mult,
                        OP.mult,
                    )
                else:
                    nc.vector.tensor_mul(
                        a_hist[:, t * BL : (t + 1) * BL], up[:], ex_sl
                    )
                u_keep[t] = up

            if t >= 5 and (t - 5) % EPOCH == 0:
                m = (t - 5) // EPOCH + 1
                if m < NEP:
                    up0 = u_keep[t]
                    nc.vector.tensor_reduce(
                        c_hist[:, m : m + 1], up0[0:1, :], AX.X, OP.add
                    )
                    cs = ev_pool.tile([1, 1], f32, tag="cs")
                    nc.vector.tensor_scalar(
                        cs[:], c_hist[:, m : m + 1], ECOMP, None, OP.mult
                    )
                    rc1 = ev_pool.tile([1, 1], f32, tag="rc1")
                    nc.vector.reciprocal(rc1[:], cs[:])
                    rc_ps = ps_misc.tile([128, 1], f32, tag="mm_small")
                    nc.tensor.matmul(rc_ps[:], onesrow[:], rc1[:], start=True, stop=True)
                    rc = ev_pool.tile([128, 1], f32, tag="rc")
                    nc.vector.tensor_copy(rc[:], rc_ps[:])
                    rc_tiles[m] = rc
            if t % 4 == 3:
                del ex4_tiles[m4]
            if t % 8 == 7:
                del x8_tiles[m8]

        # deferred M: M_e = sum_{m<=e} (ln c_m + COMP), M_0 = 0
        lnc_row = small_pool.tile([1, NEP], f32, tag="lnc_row")
        nc.scalar.activation(lnc_row[:], c_hist[:], ACTF.Ln)
        comp_row = small_pool.tile([1, NEP], f32, tag="comp_row")
        nc.vector.memset(comp_row[:], COMP)
        m_hist = small_pool.tile([1, NEP], f32, tag="m_hist")
        nc.vector.memset(m_hist[:, 0:1], 0.0)
        nc.vector.tensor_tensor_scan(
            m_hist[:, 1:NEP],
            lnc_row[:, 1:NEP],
            comp_row[:, 1:NEP],
            0.0,
            OP.add,
            OP.add,
        )

        comb_u = load_const("c_comb_u")
        comb_p = [load_const(f"c_comb_p{bl}") for bl in range(4)]
        ones16 = load_const("c_ones16")

        # ================= gold score (emitted late: fills engine gaps) ====
        for tl in range(TJ):
            nc.vector.scalar_tensor_tensor(
                junk_u[:],
                iota_k[:],
                lab4m[:, tl : tl + 1],
                data_u[:, tl * K : (tl + 1) * K],
                OP.is_equal,
                OP.mult,
                accum_out=u_acc[:, tl : tl + 1],
            )
        u_part = small_pool.tile([128, 1], f32, tag="u_part")
        nc.vector.tensor_reduce(u_part[:], u_acc[:], AX.X, OP.add)

        # per-b_local sums: view [128, 4, 512], two-level X reduce in chunks
        pair_c = small_pool.tile([128, 4 * 16], f32, tag="pair_c")
        pg3 = pair_g[:].rearrange("p (q s) -> p q s", q=4)
        for q in range(4):
            for h in range(16):
                nc.vector.tensor_reduce(
                    pair_c[:, q * 16 + h : q * 16 + h + 1],
                    pg3[:, q : q + 1, h * 32 : (h + 1) * 32],
                    AX.X,
                    OP.add,
                )
        pair_p = small_pool.tile([128, 4], f32, tag="pair_p")
        nc.vector.tensor_reduce(
            pair_p[:], pair_c[:].rearrange("p (q h) -> p q h", q=4), AX.X, OP.add
        )

        # score[b] accumulated in one [32,1] psum
        score_ps = ps_misc.tile([32, 1], f32, tag="mm_score")
        nc.tensor.matmul(score_ps[:], comb_u[:], u_part[:], start=True, stop=False)
        for bl in range(4):
            nc.tensor.matmul(
                score_ps[:],
                comb_p[bl][:],
                pair_p[:, bl : bl + 1],
                start=False,
                stop=(bl == 3),
            )
        score_sb = small_pool.tile([32, 1], f32, tag="score_sb")
        nc.vector.tensor_copy(score_sb[:], score_ps[:])

        # ================= capture logZ =================
        # quarter-gathers over bf16 pairs: idx = (L-1)*16 + b//2 within each
        # T/4 slice (clamped; out-of-range b masked later). Tile starts each
        # gather as soon as its a_hist slice is complete, overlapping the scan.
        idxp = small_pool.tile([128, 2], i32, tag="idxp")
        t0 = small_pool.tile([128, 2], i32, tag="cap_t0")
        nc.vector.tensor_scalar(t0[:], lw128[:], 16, -16, OP.mult, OP.add)
        nc.vector.tensor_tensor(idxp[:], t0[:], bhalf[:], OP.add)
        acap_q = []
        for q in range(4):
            tq = small_pool.tile([128, 2], i32, tag=f"cap_tq{q}")
            nc.vector.tensor_scalar(tq[:], idxp[:], -q * 2048, None, OP.add)
            iq = small_pool.tile([128, 2], i16, tag=f"cap_iq{q}")
            nc.vector.tensor_scalar(iq[:], tq[:], 0.0, 2047.0, OP.max, OP.min)
            aq = small_pool.tile([128, 64], bf16, tag=f"acap{q}")
            nc.gpsimd.ap_gather(
                aq[:], a_hist[:, q * 4096 : (q + 1) * 4096], iq[:],
                channels=128, num_elems=2048, d=2, num_idxs=32,
            )
            acap_q.append(aq)
        # sigma per quarter via strided ones-matmuls (even/odd b parity)
        lrow_ps = ps_misc.tile([1, 32], f32, tag="mm_small")
        nc.tensor.matmul(lrow_ps[:], seqf[:], id32[:], start=True, stop=True)
        lm1row = small_pool.tile([1, 32], f32, tag="lm1row")
        nc.vector.tensor_scalar(lm1row[:], lrow_ps[:], -1.0, None, OP.add)
        sig = small_pool.tile([1, BL], f32, tag="sig")
        nc.vector.memset(sig[:], 0.0)
        mq = small_pool.tile([1, BL], f32, tag="mq")
        mq2 = small_pool.tile([1, BL], f32, tag="mq2")
        sq = small_pool.tile([1, BL], f32, tag="sq")
        for q in range(4):
            sg_ev = ps_misc.tile([1, 16], f32, tag="mm_small")
            nc.tensor.matmul(
                sg_ev[:], onescol_bf[:], acap_q[q][:, 0:64:4], start=True, stop=True
            )
            sg_od = ps_misc.tile([1, 16], f32, tag="mm_score")
            nc.tensor.matmul(
                sg_od[:], onescol_bf[:], acap_q[q][:, 3:64:4], start=True, stop=True
            )
            nc.vector.tensor_copy(sq[:, 0:32:2], sg_ev[:])
            nc.vector.tensor_copy(sq[:, 1:32:2], sg_od[:])
            # in-range mask: q*128 <= L-1 < (q+1)*128
            nc.vector.tensor_scalar(mq[:], lm1row[:], float(q * 128), None, OP.is_ge)
            nc.vector.tensor_scalar(
                mq2[:], lm1row[:], float((q + 1) * 128), None, OP.is_lt
            )
            nc.vector.tensor_tensor(mq[:], mq[:], mq2[:], OP.mult)
            nc.vector.tensor_tensor(sq[:], sq[:], mq[:], OP.mult)
            nc.vector.tensor_tensor(sig[:], sig[:], sq[:], OP.add)
        lz = small_pool.tile([1, BL], f32, tag="lz")
        nc.scalar.activation(lz[:], sig[:], ACTF.Ln)

        # M expanded per step (M_step[t] = M_hist[t//4]) then gathered at t=L-1
        m_step_row = small_pool.tile([1, T], f32, tag="m_step_row")
        for r in range(EPOCH):
            nc.vector.tensor_copy(m_step_row[:, r::EPOCH], m_hist[:])
        m16_ps = ps_misc.tile([16, T], f32, tag="mm_score")
        nc.tensor.matmul(m16_ps[:], ones16[:], m_step_row[:], start=True, stop=True)
        m_step16 = small_pool.tile([16, T], f32, tag="m_step16")
        nc.vector.tensor_copy(m_step16[:], m16_ps[:])
        lm1_16 = small_pool.tile([16, 2], i16, tag="lm1_16")
        nc.vector.tensor_scalar(lm1_16[:], lw128[0:16, :], -1.0, None, OP.add)
        mcap = small_pool.tile([16, BL], f32, tag="mcap")
        nc.gpsimd.ap_gather(
            mcap[:], m_step16[:], lm1_16[:], channels=16, num_elems=T, d=1, num_idxs=32
        )
        lzm = small_pool.tile([1, BL], f32, tag="lzm")
        nc.vector.tensor_add(lzm[:], lz[:], mcap[0:1, :])

        # ================= final loss =================
        scT_ps = ps_misc.tile([1, 32], f32, tag="mm_small")
        nc.tensor.matmul(scT_ps[:], score_sb[:], id32[:], start=True, stop=True)
        diff = small_pool.tile([1, BL], f32, tag="diff")
        nc.vector.tensor_sub(diff[:], lzm[:], scT_ps[:])
        loss_sb = small_pool.tile([1, 1], f32, tag="loss_sb")
        nc.vector.tensor_reduce(loss_sb[:], diff[:], AX.X, OP.add)
        nc.sync.dma_start(loss_d[:, :], loss_sb[:])

    nc.compile()
    return nc, consts_np


def _get_program():
    if "prog" not in _CACHE:
        _CACHE["prog"] = _build_program()
    return _CACHE["prog"]


def kernel(logits, labels, seq_lens, trans):
    from concourse.bass_utils import run_bass_kernel_spmd

    nc, consts_np = _get_program()
    logits = np.asarray(logits, dtype=np.float32)
    labels = np.asarray(labels, dtype=np.int32)
    seq_lens = np.asarray(seq_lens, dtype=np.int32)
    trans = np.asarray(trans, dtype=np.float32)

    in_maps = []
    for c in range(NCORES):
        sl = slice(c * BL, (c + 1) * BL)
        m = {
            "logits": np.ascontiguousarray(logits[sl]),
            "labels": np.ascontiguousarray(labels[sl]),
            "seq_lens": np.ascontiguousarray(seq_lens[sl]),
            "trans": trans,
        }
        m.update(consts_np)
        in_maps.append(m)

    res = run_bass_kernel_spmd(nc, in_maps, list(range(NCORES)))
    total = sum(float(res.results[c]["loss"][0, 0]) for c in range(NCORES))
    return np.float32(total)

